# revision 9
# baseline (speedup 1.0000x reference)
"""EnsembleGRU Trainium2 kernel, v2: PE-assisted scan.

Math (per ensemble member e, H=1):
    gi = x @ Wc^T + bc   (Wc = Wih @ Wl folded on host; bc likewise)
    scan over W steps:
        r  = sigmoid(gi_r + a*h)            a = whh[0]
        z  = sigmoid(gi_z + b*h)            b = whh[1]
        n  = tanh(gi_n + r*(c*h + d))       c = whh[2], d = bhh[2]
        h' = (1-z)*n + z*h = q - u,  q = z*h, u = (z-1)*n

v2 scan structure: the gate-arg constructions run on the TensorEngine as
tiny accumulating diag matmuls into the PSUM gi regions:
    gi_r[w+1] += diag(a)*q(w)  (early, after sigma)
    gi_r[w+1] += diag(-a)*u(w) (late, after tanh)   => + a*h'(w)
    gi_n[w]   += diag(c)*v(w),  v = h*r
so sigma/tanh read finished args straight from PSUM and the DVE only does
v, q, u, h' per step. Group matmuls are sliced across scan steps to keep
the in-order PE queue from blocking the latency-critical assist matmuls.

Sharding: E=16 members over 8 cores (2 per core), zero communication.
Lane layout per core: partition p = e_loc*64 + p' (p' in 0..63),
free col c in 0..39, bi = p'*40 + c  (5120 lanes = 128 x 40).
"""

import numpy as np

W, E, B, I, F = 64, 16, 256, 10, 8
BI = B * I            # 2560
NCORES = 8
E_LOC = E // NCORES   # 2
PP = 64               # partitions per member
CC = BI // PP         # 40 free cols per step
G = 3                 # gates

WG = 8                # steps per gi matmul group
NGRP = W // WG
# diag slot layout, grouped to match the head DMA slices:
#   [r: bias,f0-7 | z: bias,f0-7 | a, b | n: bias,f0-7 | -a, -b, c]
D_A, D_B, D_NA, D_NB, D_C = 18, 19, 29, 30, 31
NDIAG = 32
_DG_BASE = (0, 9, 20)


def _dslot(g, f):
    """diag slot for gate g, f-term f (f == -1 -> bias)."""
    return _DG_BASE[g] + 1 + f

_CACHED = {}


def _build_nc(d_nonzero: bool, rep: int = 1, unroll: bool = False):
    import contextlib

    import concourse.bacc as bacc
    import concourse.mybir as mybir
    from concourse.tile import TileContext

    AL = mybir.AluOpType
    AF = mybir.ActivationFunctionType
    f32 = mybir.dt.float32
    f16 = mybir.dt.float16

    nc = bacc.Bacc("TRN2", target_bir_lowering=False)

    xh = nc.dram_tensor("xh", [128, F, W, CC], f16, kind="ExternalInput")
    dg = nc.dram_tensor("dg", [128, NDIAG * 128], f16, kind="ExternalInput")
    cst = nc.dram_tensor("cst", [128, 7 + CC + F], f32, kind="ExternalInput")
    out = nc.dram_tensor("out", [128, W * CC], f32, kind="ExternalOutput")

    with TileContext(nc) as tc:
        with (
            tc.tile_pool(name="const", bufs=1) as constp,
            tc.tile_pool(name="xp", bufs=2) as xp,
            tc.tile_pool(name="gip", bufs=2, space="PSUM") as gip,
            tc.tile_pool(name="ginp", bufs=2) as ginp,
            tc.tile_pool(name="gin0p", bufs=2) as gin0p,
            tc.tile_pool(name="warmp", bufs=1, space="PSUM") as warmp,
            tc.tile_pool(name="scan", bufs=3) as scanp,
            tc.tile_pool(name="outp", bufs=1) as outp,
        ):
            dg_sb = constp.tile([128, NDIAG * 128], f16, tag="dg")
            cst_sb = constp.tile([128, 7 + CC + F], f32, tag="cst")
            ones = constp.tile([128, WG * CC], f16, tag="ones")
            h0h = constp.tile([128, CC], f16, tag="h0h")
            out_sb = outp.tile([128, (W + 1) * CC], f32, tag="out")

            # head DMAs in explicit consumption order on one queue (the
            # DMA engines serialize transfers anyway): cst, r diags, x0
            # (emitted in _body), z+ab diags, n+rest diags, x1
            nc.sync.dma_start(cst_sb[:], cst[:])
            nc.sync.dma_start(dg_sb[:, : 9 * 128], dg[:, : 9 * 128])
            nc.vector.memset(ones[:], 1.0)
            # h0 into slot 0 (f32) and fp16 copy for the PE assists
            nc.vector.tensor_copy(out_sb[:, 0:CC], cst_sb[:, 7 : 7 + CC])
            nc.vector.tensor_copy(h0h[:], cst_sb[:, 7 : 7 + CC])

            c_s = cst_sb[:, 2:3]
            d_s = cst_sb[:, 3:4]

            # n-gate bias broadcast tile (ACT Copy with per-partition scale)
            bcn = constp.tile([128, WG * CC], f32, tag="bcn")
            nc.scalar.mul(bcn[:], ones[:], cst_sb[:, 4:5])

            # PE p-state warm-up: contiguous dummy matmuls during the input
            # DMA window so group 0's mains run at full clock
            warm = warmp.tile([128, 512], f32, tag="warm")
            for _ in range(10):
                nc.tensor.matmul(warm[:, : 5 * CC], ones[:, :128],
                                 ones[:, : 5 * CC], start=True, stop=True,
                                 skip_group_check=True)

            if unroll and rep > 1:
                for _ in range(rep):
                    _body(
                        nc, tc, xp, gip, ginp, scanp, xh, out, dg, dg_sb, ones,
                        h0h, bcn, out_sb, cst_sb, c_s, d_s, AL, AF, f32, f16,
                        d_nonzero, gin0p,
                    )
            else:
                loop_cm = (tc.For_i(0, rep, 1) if rep > 1
                           else contextlib.nullcontext())
                with loop_cm:
                    _body(
                        nc, tc, xp, gip, ginp, scanp, xh, out, dg, dg_sb, ones,
                        h0h, bcn, out_sb, cst_sb, c_s, d_s, AL, AF, f32, f16,
                        d_nonzero, gin0p,
                    )

    nc.finalize()
    return nc


def _body(
    nc, tc, xp, gip, ginp, scanp, xh, out, dg, dg_sb, ones, h0h, bcn, out_sb,
    cst_sb, c_s, d_s, AL, AF, f32, f16, d_nonzero, gin0p,
):
    def diag(i):
        return dg_sb[:, i * 128 : (i + 1) * 128]

    gi_tiles = {}
    gin_tiles = {}
    x_tiles = {}

    def emit_x_dma(k):
        x_t = xp.tile([128, F * WG * CC], f16, tag="x")
        x_tiles[k] = x_t
        nc.sync.dma_start(
            x_t[:].rearrange("p (f w c) -> p f w c", f=F, c=CC),
            xh[:, :, k * WG : (k + 1) * WG, :],
        )

    def alloc_gi(k):
        gi_tiles[k] = gip.tile([128, 3 * 512], f32, tag="gi", name="gi")
        gin_tiles[k] = ginp.tile([128, WG * CC], f16, tag="gin", name="gin")

    # one main-matmul slice: (g, f) with f == -1 meaning the bias matmul
    def emit_main_mm(k, g, f):
        gi_ps = gi_tiles[k]
        reg = gi_ps[:, g * 512 : g * 512 + WG * CC]
        if f < 0:
            nc.tensor.matmul(
                reg, diag(_dslot(g, -1)), ones[:, : WG * CC],
                start=True, stop=False, skip_group_check=True,
            )
        else:
            nc.tensor.matmul(
                reg, diag(_dslot(g, f)),
                x_tiles[k][:, f * WG * CC : (f + 1) * WG * CC],
                start=False, stop=(f == F - 1), skip_group_check=True,
            )

    # main-mm schedule: region-major (r fully, then z, then n) so the
    # earliest-read regions complete first when sliced across steps
    RZ_SEQ = [(g, f) for g in range(2) for f in [-1] + list(range(F))]
    N_SEQ = [(2, f) for f in [-1] + list(range(F))]
    # slices emit the n gate first so the gin copy can run mid-group,
    # far from the boundary sigma
    MAIN_SEQ = N_SEQ + RZ_SEQ
    NMAIN = len(MAIN_SEQ)  # 27

    def gi_ap(w, g):
        k, wl = divmod(w, WG)
        return gi_tiles[k][:, g * 512 + wl * CC : g * 512 + (wl + 1) * CC]

    def gi_rz_ap(w):
        k, wl = divmod(w, WG)
        t = gi_tiles[k][:]
        return t.rearrange("p (g x) -> p g x", g=3)[:, 0:2, wl * CC : (wl + 1) * CC]

    def gin_ap(w):
        k, wl = divmod(w, WG)
        return gin_tiles[k][:, wl * CC : (wl + 1) * CC]

    # n-gate gi: PE mains accumulate it in PSUM as usual, then one ACT
    # copy per group moves it to SBUF so the an stt chains in-engine
    # after v with no PE/PSUM round-trip on the critical path
    def emit_gin_copy(k):
        # gin = gi_n / c (fp16): an = v + gin is then a 2x tt, and tanh
        # rescales by c via its per-partition input scale
        nc.scalar.mul(gin_tiles[k][:],
                      gi_tiles[k][:, 2 * 512 : 2 * 512 + WG * CC],
                      cst_sb[:, 5:6])

    def emit_out_dma(k):
        nc.sync.dma_start(
            out[:, k * WG * CC : (k + 1) * WG * CC],
            out_sb[:, (k * WG + 1) * CC : ((k + 1) * WG + 1) * CC],
        )

    # prologue: group 0 mains fully + h0 assists; group k>=1 mains are
    # sliced across group k-1's steps (1-group lookahead: the PSUM buf WAR
    # vs group k-2 is already clear, so slices run in each step's PE idle
    # instead of bursting at the boundary).
    emit_x_dma(0)
    # remaining diag slices, ordered between the x loads by consumption
    # time (z+ab right after x0 so z mains can start; n before x1 so
    # tanh(0) isn't gated by the bigger x1 transfer)
    nc.sync.dma_start(dg_sb[:, 9 * 128 : 20 * 128], dg[:, 9 * 128 : 20 * 128])
    nc.sync.dma_start(dg_sb[:, 20 * 128 :], dg[:, 20 * 128 :])
    emit_x_dma(1)
    alloc_gi(0)
    HCOL = WG * CC // 2

    def emit_main_half0(g, f, hh):
        # PSUM start=True resets the whole bank, so the bias matmul is
        # emitted full-width once (hh==0) and the halves only split the
        # f-term matmuls
        if f < 0:
            if hh == 0:
                nc.tensor.matmul(gi_tiles[0][:, g * 512 : g * 512 + WG * CC],
                                 diag(_dslot(g, -1)), ones[:, : WG * CC],
                                 start=True, stop=False, skip_group_check=True)
            return
        reg = gi_tiles[0][:, g * 512 + hh * HCOL : g * 512 + hh * HCOL + HCOL]
        base = f * WG * CC + hh * HCOL
        nc.tensor.matmul(reg, diag(_dslot(g, f)),
                         x_tiles[0][:, base : base + HCOL],
                         start=False, stop=(f == F - 1),
                         skip_group_check=True)

    # r/z half mains (all sigma(0) needs) + h0 assists; b-halves and the
    # n mains are emitted inside step 0 so tile deps don't stall sigma(0)
    for g, f in RZ_SEQ:
        emit_main_half0(g, f, 0)
    nc.tensor.matmul(gi_ap(0, 0), diag(D_A), h0h[:], start=False, stop=True,
                     skip_group_check=True)
    nc.tensor.matmul(gi_ap(0, 1), diag(D_B), h0h[:], start=False, stop=True,
                     skip_group_check=True)

    # front-loaded slice sizes per step of the previous group
    SLICE_SIZES = [4, 4, 4, 4, 4, 3, 2, 2]
    SLICE_LO = [sum(SLICE_SIZES[:i]) for i in range(WG + 1)]

    def slice_for_step(w):
        k, wl = divmod(w, WG)
        kk = k + 1
        if kk >= NGRP:
            return []
        return [(kk, g, f) for (g, f) in MAIN_SEQ[SLICE_LO[wl]:SLICE_LO[wl + 1]]]

    for w in range(W):
        k, wl = divmod(w, WG)
        if w > 0 and wl == 0:
            emit_out_dma(k - 1)
        if wl == 0:
            if k + 1 < NGRP:
                alloc_gi(k + 1)
            if k + 2 < NGRP:
                emit_x_dma(k + 2)

        h = h0h if w == 0 else h16_prev
        rz = scanp.tile([128, 2 * CC], f16, tag="rz")
        n_t = scanp.tile([128, CC], f16, tag="n")
        v = scanp.tile([128, CC], f16, tag="v")
        q = scanp.tile([128, CC], f16, tag="q")
        u = scanp.tile([128, CC], f16, tag="u")
        h16 = scanp.tile([128, CC], f16, tag="h16")

        # r|z = sigmoid(gi_r + a*h | gi_z + b*h)  (args finished in PSUM)
        nc.scalar.activation(rz[:].rearrange("p (g x) -> p g x", g=2),
                             gi_rz_ap(w), AF.Sigmoid)
        if w == 0:
            # n mains first (tanh(0) needs the copy), then the r/z b-halves
            for g, f in N_SEQ:
                emit_main_mm(0, g, f)
            emit_gin_copy(0)
            for g, f in RZ_SEQ:
                emit_main_half0(g, f, 1)
        # v = h*r, then an = c*v + gi_n: both DVE (in-engine chaining, no
        # PE round-trip)  [+ (d/c)*r folded into v if d != 0]
        nc.vector.tensor_tensor(v[:], h, rz[:, 0:CC], AL.mult)
        if d_nonzero:
            nc.vector.scalar_tensor_tensor(v[:], rz[:, 0:CC], d_s, v[:],
                                           AL.mult, AL.add)
        an = scanp.tile([128, CC], f16, tag="an")
        nc.vector.tensor_tensor(an[:], v[:], gin_ap(w), AL.add)
        # q = z*h (feeds early assists for w+1)
        nc.vector.tensor_tensor(q[:], rz[:, CC:], h, AL.mult)
        # n = tanh(an)
        # (early assists emitted after tanh: tile-granular dep tracking would
        # otherwise stall tanh on them)
        nc.scalar.activation(n_t[:], an[:], AF.Tanh, scale=c_s)
        if w + 1 < W:
            nc.tensor.matmul(gi_ap(w + 1, 0), diag(D_A), q[:], start=False,
                             stop=True, skip_group_check=True)
            nc.tensor.matmul(gi_ap(w + 1, 1), diag(D_B), q[:], start=False,
                             stop=True, skip_group_check=True)
        # u = (z-1)*n = s - n with s = z*n; the -a*u late assist splits
        # into +a*n (ready at tanh, off-chain) and -a*s
        if w + 1 < W:
            nc.tensor.matmul(gi_ap(w + 1, 0), diag(D_A), n_t[:], start=False,
                             stop=True, skip_group_check=True)
            nc.tensor.matmul(gi_ap(w + 1, 1), diag(D_B), n_t[:], start=False,
                             stop=True, skip_group_check=True)
        nc.vector.tensor_tensor(u[:], rz[:, CC:], n_t[:], AL.mult)  # s = z*n
        if w + 1 < W:
            nc.tensor.matmul(gi_ap(w + 1, 0), diag(D_NA), u[:], start=False,
                             stop=True, skip_group_check=True)
            nc.tensor.matmul(gi_ap(w + 1, 1), diag(D_NB), u[:], start=False,
                             stop=True, skip_group_check=True)
        # h' = q - s + n: fp16 copy for the scan chain, f32 for the output
        t_h = scanp.tile([128, CC], f16, tag="t_h")
        nc.vector.tensor_tensor(t_h[:], q[:], u[:], AL.subtract)
        nc.vector.tensor_tensor(h16[:], t_h[:], n_t[:], AL.add)
        nc.vector.tensor_tensor(out_sb[:, (w + 1) * CC : (w + 2) * CC],
                                t_h[:], n_t[:], AL.add)
        h16_prev = h16
        for (kk, g, f) in slice_for_step(w):
            emit_main_mm(kk, g, f)
        if wl == 2 and k + 1 < NGRP:
            emit_gin_copy(k + 1)


    emit_out_dma(NGRP - 1)


# v is c-folded: fold c into the v stt; diag(D_C) stays ones so the PSUM
# accumulate adds v as-is.  (D_C diag values are set to 1.0 host-side.)


def _prep_core_inputs(inputs, core):
    x = inputs["inputs"]          # (W,E,B,I,F) f32
    state = inputs["state"]       # (1,E,BI,1)
    wl = inputs["weight_linear"]  # (E,16,F)
    bl = inputs["bias_linear"]    # (E,16)
    wih = inputs["weight_ih"]     # (E,3,16)
    whh = inputs["weight_hh"]     # (E,3,1)
    bih = inputs["bias_ih"]       # (E,3)
    bhh = inputs["bias_hh"]       # (E,3)

    es = slice(core * E_LOC, (core + 1) * E_LOC)
    Wc = np.einsum("egp,epf->egf", wih[es], wl[es])          # (2,3,F)
    bc = np.einsum("egp,ep->eg", wih[es], bl[es]) + bih[es]  # (2,3)
    bc = bc.copy()
    bc[:, 0] += bhh[es][:, 0]
    bc[:, 1] += bhh[es][:, 1]
    # n-gate hh bias (d) is handled in the scan when nonzero

    xr = np.asarray(x[:, es]).reshape(W, E_LOC, PP, CC, F)
    xh = np.ascontiguousarray(xr.transpose(1, 2, 4, 0, 3)).reshape(128, F, W, CC)
    xh = xh.astype(np.float16)

    pe = np.repeat(np.arange(E_LOC), PP)  # (128,) member index per partition
    dgv = np.zeros((128, NDIAG), np.float32)
    for g in range(G):
        for f in range(F):
            dgv[:, _dslot(g, f)] = Wc[pe, g, f]
        dgv[:, _dslot(g, -1)] = bc[pe, g]
    a = whh[es][pe, 0, 0]
    b = whh[es][pe, 1, 0]
    dgv[:, D_A] = a
    dgv[:, D_B] = b
    dgv[:, D_NA] = -a
    dgv[:, D_NB] = -b
    dgv[:, D_C] = whh[es][pe, 2, 0]
    dgm = np.zeros((128, NDIAG, 128), np.float16)
    idx = np.arange(128)
    dgm[idx, :, idx] = dgv.astype(np.float16)
    dgm = dgm.reshape(128, NDIAG * 128)

    cstv = np.zeros((128, 7 + CC + F), np.float32)
    cstv[:, 0] = a
    cstv[:, 1] = b
    cstv[:, 2] = whh[es][pe, 2, 0]
    with np.errstate(divide="ignore", invalid="ignore"):
        cstv[:, 3] = np.where(cstv[:, 2] != 0, bhh[es][pe, 2] / cstv[:, 2], 0.0)
    cstv[:, 4] = bc[pe, 2]
    cstv[:, 5] = 1.0 / cstv[:, 2]
    cstv[:, 6] = -b
    h0 = np.asarray(state[-1, es, :, 0]).reshape(E_LOC, PP, CC)
    cstv[:, 7 : 7 + CC] = h0.reshape(128, CC)
    for f in range(F):
        cstv[:, 7 + CC + f] = Wc[pe, 2, f]

    return {"xh": xh, "dg": dgm, "cst": cstv}


def kernel(**inputs):
    from concourse.bass_utils import run_bass_kernel_spmd

    bhh = np.asarray(inputs["bias_hh"])
    d_nonzero = bool(np.any(bhh[:, 2] != 0))

    key = ("nc", d_nonzero)
    if key not in _CACHED:
        _CACHED[key] = _build_nc(d_nonzero)
    nc = _CACHED[key]

    in_maps = [_prep_core_inputs(inputs, c) for c in range(NCORES)]
    res = run_bass_kernel_spmd(nc, in_maps, core_ids=list(range(NCORES)))

    full = np.zeros((W, E, B, I, 1), np.float32)
    for c in range(NCORES):
        o = np.asarray(res.results[c]["out"]).reshape(E_LOC, PP, W, CC)
        o = o.transpose(2, 0, 1, 3).reshape(W, E_LOC, BI)
        full[:, c * E_LOC : (c + 1) * E_LOC] = o.reshape(W, E_LOC, B, I, 1)
    return full
